# revision 11
# baseline (speedup 1.0000x reference)
"""Embedding lookup (gather) on 8 Trainium2 NeuronCores.

Full inputs: input_ids [8, 4096] int32, weight [128000, 1024] f32.
Output: weight[input_ids] -> [8, 4096, 1024] f32.

Strategy: data-parallel over tokens. Core b handles batch row b
(4096 tokens). Each core receives the full weight table (host->HBM
staging is not timed) plus its ids reshaped to [128, 32] (token
p*32+g at ids[p, g]). On-device: 32 indirect-DMA gathers (one index
per partition each -- the HW DGE contract -- pulling weight rows
HBM -> SBUF), grouped into K store groups; each group's [128, m*1024]
tile is stored with one fully partition-contiguous DMA to the output.
Gathers run on gpsimd (SWDGE), stores on sync (HWDGE), overlapped via
per-group semaphores. No collectives needed.

Raw Bass (no TileContext): this walrus build rejects any instruction
carrying more than one sem-wait command, so waits are standalone
sequencer instructions; all sem waits are exact-total thresholds.
"""

from contextlib import ExitStack

import numpy as np

from concourse import bass, mybir
from concourse.bass_utils import run_bass_kernel_spmd

VOCAB = 128000
DIM = 1024
BATCH = 8
SEQ = 4096
N_CORES = 8
P = 128

Q = SEQ // P  # tokens per partition = gather ops per core (32)
K = 16  # store groups
M = Q // K  # gathers per store group


def _build_nc(vocab=VOCAB, dim=DIM, seq=SEQ, k_groups=K):
    q = seq // P
    m = q // k_groups
    nc = bass.Bass()
    ids = nc.declare_dram_parameter("ids", [P, q], mybir.dt.int32, isOutput=False)
    weight = nc.declare_dram_parameter(
        "weight", [vocab, dim], mybir.dt.float32, isOutput=False
    )
    out = nc.declare_dram_parameter("out", [seq, dim], mybir.dt.float32, isOutput=True)
    # Output viewed per-partition: partition p's tokens are rows
    # [p*q, (p+1)*q), i.e. one contiguous q*dim chunk per partition.
    out_pview = out[:].rearrange("(p q) d -> p (q d)", p=P)

    with ExitStack() as ctx:
        ids_tile = ctx.enter_context(nc.sbuf_tensor("ids_tile", [P, q], mybir.dt.int32))
        tiles = [
            ctx.enter_context(
                nc.sbuf_tensor(f"grp{k}", [P, m * dim], mybir.dt.float32)
            )
            for k in range(k_groups)
        ]
        ids_sem = ctx.enter_context(nc.semaphore("ids_sem"))
        gsems = [ctx.enter_context(nc.semaphore(f"gsem{k}")) for k in range(k_groups)]
        out_sem = ctx.enter_context(nc.semaphore("out_sem"))
        block = ctx.enter_context(nc.Block())

        @block.gpsimd
        def _(g):
            g.dma_start(out=ids_tile[:], in_=ids[:]).then_inc(ids_sem, 16)
            g.wait_ge(ids_sem, 16)
            for j in range(q):
                k = j // m
                g.indirect_dma_start(
                    out=tiles[k][:, (j - k * m) * dim : (j - k * m + 1) * dim],
                    out_offset=None,
                    in_=weight[:],
                    in_offset=bass.IndirectOffsetOnAxis(
                        ap=ids_tile[:, j : j + 1], axis=0
                    ),
                ).then_inc(gsems[k], 16)

        @block.sync
        def _(s):
            for k in range(k_groups):
                # All m gathers of group k done (exact total: m*16 incs).
                s.wait_ge(gsems[k], 16 * m)
                s.dma_start(
                    out=out_pview[:, k * m * dim : (k + 1) * m * dim],
                    in_=tiles[k][:],
                ).then_inc(out_sem, 16)
            s.wait_ge(out_sem, 16 * k_groups)

    return nc


def _make_in_maps(input_ids: np.ndarray, weight: np.ndarray):
    input_ids = np.asarray(input_ids)
    weight = np.ascontiguousarray(np.asarray(weight, dtype=np.float32))
    seq = input_ids.shape[1]
    q = seq // P
    in_maps = []
    for b in range(input_ids.shape[0]):
        ids_r = np.ascontiguousarray(input_ids[b].astype(np.int32).reshape(P, q))
        in_maps.append({"ids": ids_r, "weight": weight})
    return in_maps


def kernel(input_ids: np.ndarray, weight: np.ndarray) -> np.ndarray:
    input_ids = np.asarray(input_ids)
    B, S = input_ids.shape
    assert (B, S) == (BATCH, SEQ)

    nc = _build_nc()
    in_maps = _make_in_maps(input_ids, weight)
    res = run_bass_kernel_spmd(nc, in_maps, list(range(N_CORES)))
    return np.stack([res.results[b]["out"] for b in range(B)], axis=0)


# revision 13
# speedup vs baseline: 1.1133x; 1.1133x over previous
"""Embedding lookup (gather) on 8 Trainium2 NeuronCores.

Full inputs: input_ids [8, 4096] int32, weight [128000, 1024] f32.
Output: weight[input_ids] -> [8, 4096, 1024] f32.

Strategy: data-parallel over tokens. Core b handles batch row b
(4096 tokens). Each core receives the full weight table (host->HBM
staging is not timed) plus its ids reshaped to [128, 32] (token
p*32+g at ids[p, g]). On-device: 32 indirect-DMA gathers (one index
per partition each -- the HW DGE contract -- pulling weight rows
HBM -> SBUF), grouped into K store groups; each group's [128, m*1024]
tile is stored with one fully partition-contiguous DMA to the output.
Gathers run on gpsimd (SWDGE), stores on sync (HWDGE), overlapped via
per-group semaphores. No collectives needed.

Raw Bass (no TileContext): this walrus build rejects any instruction
carrying more than one sem-wait command, so waits are standalone
sequencer instructions; all sem waits are exact-total thresholds.
"""

from contextlib import ExitStack

import numpy as np

from concourse import bass, mybir
from concourse.bass_utils import run_bass_kernel_spmd

VOCAB = 128000
DIM = 1024
BATCH = 8
SEQ = 4096
N_CORES = 8
P = 128

Q = SEQ // P  # tokens per partition = gather ops per core (32)
# Store-group sizes (in gathers): small first groups start the write
# stream early; a tiny last group keeps the drain tail short.
GROUPS = (1, 1, 2, 2, 4, 4, 4, 4, 4, 4, 1, 1)
assert sum(GROUPS) == Q


def _build_nc(vocab=VOCAB, dim=DIM, seq=SEQ, groups=GROUPS):
    q = seq // P
    assert sum(groups) == q
    nc = bass.Bass()
    ids = nc.declare_dram_parameter("ids", [P, q], mybir.dt.int32, isOutput=False)
    weight = nc.declare_dram_parameter(
        "weight", [vocab, dim], mybir.dt.float32, isOutput=False
    )
    out = nc.declare_dram_parameter("out", [seq, dim], mybir.dt.float32, isOutput=True)
    # Output viewed per-partition: partition p's tokens are rows
    # [p*q, (p+1)*q), i.e. one contiguous q*dim chunk per partition.
    out_pview = out[:].rearrange("(p q) d -> p (q d)", p=P)

    k_groups = len(groups)
    starts = [sum(groups[:k]) for k in range(k_groups)]  # first gather of group k

    with ExitStack() as ctx:
        ids_tile = ctx.enter_context(nc.sbuf_tensor("ids_tile", [P, q], mybir.dt.int32))
        tiles = [
            ctx.enter_context(
                nc.sbuf_tensor(f"grp{k}", [P, groups[k] * dim], mybir.dt.float32)
            )
            for k in range(k_groups)
        ]
        ids_sem = ctx.enter_context(nc.semaphore("ids_sem"))
        gsems = [ctx.enter_context(nc.semaphore(f"gsem{k}")) for k in range(k_groups)]
        out_sem = ctx.enter_context(nc.semaphore("out_sem"))
        block = ctx.enter_context(nc.Block())

        @block.gpsimd
        def _(g):
            g.dma_start(out=ids_tile[:], in_=ids[:]).then_inc(ids_sem, 16)
            g.wait_ge(ids_sem, 16)
            for k in range(k_groups):
                for i in range(groups[k]):
                    j = starts[k] + i
                    g.indirect_dma_start(
                        out=tiles[k][:, i * dim : (i + 1) * dim],
                        out_offset=None,
                        in_=weight[:],
                        in_offset=bass.IndirectOffsetOnAxis(
                            ap=ids_tile[:, j : j + 1], axis=0
                        ),
                    ).then_inc(gsems[k], 16)

        @block.sync
        def _(s):
            for k in range(k_groups):
                # All gathers of group k done (exact total: groups[k]*16 incs).
                s.wait_ge(gsems[k], 16 * groups[k])
                s.dma_start(
                    out=out_pview[:, starts[k] * dim : (starts[k] + groups[k]) * dim],
                    in_=tiles[k][:],
                ).then_inc(out_sem, 16)
            s.wait_ge(out_sem, 16 * k_groups)

    return nc


def _make_in_maps(input_ids: np.ndarray, weight: np.ndarray):
    input_ids = np.asarray(input_ids)
    weight = np.ascontiguousarray(np.asarray(weight, dtype=np.float32))
    seq = input_ids.shape[1]
    q = seq // P
    in_maps = []
    for b in range(input_ids.shape[0]):
        ids_r = np.ascontiguousarray(input_ids[b].astype(np.int32).reshape(P, q))
        in_maps.append({"ids": ids_r, "weight": weight})
    return in_maps


def kernel(input_ids: np.ndarray, weight: np.ndarray) -> np.ndarray:
    input_ids = np.asarray(input_ids)
    B, S = input_ids.shape
    assert (B, S) == (BATCH, SEQ)

    nc = _build_nc()
    in_maps = _make_in_maps(input_ids, weight)
    res = run_bass_kernel_spmd(nc, in_maps, list(range(N_CORES)))
    return np.stack([res.results[b]["out"] for b in range(B)], axis=0)


# revision 15
# speedup vs baseline: 1.1272x; 1.0125x over previous
"""Embedding lookup (gather) on 8 Trainium2 NeuronCores.

Full inputs: input_ids [8, 4096] int32, weight [128000, 1024] f32.
Output: weight[input_ids] -> [8, 4096, 1024] f32.

Strategy: data-parallel over tokens. Core b handles batch row b
(4096 tokens). Each core receives the full weight table (host->HBM
staging is not timed) plus its ids reshaped to [128, 32] (token
p*32+g at ids[p, g]). On-device: 32 indirect-DMA gathers (one index
per partition each -- the HW DGE contract -- pulling weight rows
HBM -> SBUF), grouped into K store groups; each group's [128, m*1024]
tile is stored with one fully partition-contiguous DMA to the output.
Gathers run on gpsimd (SWDGE), stores on sync (HWDGE), overlapped via
per-group semaphores. No collectives needed.

Raw Bass (no TileContext): this walrus build rejects any instruction
carrying more than one sem-wait command, so waits are standalone
sequencer instructions; all sem waits are exact-total thresholds.
"""

from contextlib import ExitStack

import numpy as np

from concourse import bass, mybir
from concourse.bass_utils import run_bass_kernel_spmd

VOCAB = 128000
DIM = 1024
BATCH = 8
SEQ = 4096
N_CORES = 8
P = 128

Q = SEQ // P  # tokens per partition = gather ops per core (32)
# Store-group sizes (in gathers): small first groups start the write
# stream early; a tiny last group keeps the drain tail short.
GROUPS = (1, 1, 2, 2, 4, 4, 4, 4, 4, 4, 1, 1)
assert sum(GROUPS) == Q


def _build_nc(vocab=VOCAB, dim=DIM, seq=SEQ, groups=GROUPS):
    q = seq // P
    assert sum(groups) == q
    nc = bass.Bass()
    ids = nc.declare_dram_parameter("ids", [P, q], mybir.dt.int32, isOutput=False)
    weight = nc.declare_dram_parameter(
        "weight", [vocab, dim], mybir.dt.float32, isOutput=False
    )
    out = nc.declare_dram_parameter("out", [seq, dim], mybir.dt.float32, isOutput=True)
    # Output viewed per-partition: partition p's tokens are rows
    # [p*q, (p+1)*q), i.e. one contiguous q*dim chunk per partition.
    out_pview = out[:].rearrange("(p q) d -> p (q d)", p=P)

    k_groups = len(groups)
    starts = [sum(groups[:k]) for k in range(k_groups)]  # first gather of group k

    with ExitStack() as ctx:
        ids_tile = ctx.enter_context(nc.sbuf_tensor("ids_tile", [P, q], mybir.dt.int32))
        tiles = [
            ctx.enter_context(
                nc.sbuf_tensor(f"grp{k}", [P, groups[k] * dim], mybir.dt.float32)
            )
            for k in range(k_groups)
        ]
        ids_sem = ctx.enter_context(nc.semaphore("ids_sem"))
        gsems = [ctx.enter_context(nc.semaphore(f"gsem{k}")) for k in range(k_groups)]
        out_sem = ctx.enter_context(nc.semaphore("out_sem"))
        block = ctx.enter_context(nc.Block())

        @block.gpsimd
        def _(g):
            g.wait_ge(ids_sem, 16)
            for k in range(k_groups):
                for i in range(groups[k]):
                    j = starts[k] + i
                    g.indirect_dma_start(
                        out=tiles[k][:, i * dim : (i + 1) * dim],
                        out_offset=None,
                        in_=weight[:],
                        in_offset=bass.IndirectOffsetOnAxis(
                            ap=ids_tile[:, j : j + 1], axis=0
                        ),
                    ).then_inc(gsems[k], 16)

        @block.sync
        def _(s):
            # HWDGE ids load: Sync's preamble finishes earlier than GpSimd's
            # and HWDGE first-byte latency is lower, so the gathers unblock
            # sooner than with a gpsimd-issued load.
            s.dma_start(out=ids_tile[:], in_=ids[:]).then_inc(ids_sem, 16)
            for k in range(k_groups):
                # All gathers of group k done (exact total: groups[k]*16 incs).
                s.wait_ge(gsems[k], 16 * groups[k])
                s.dma_start(
                    out=out_pview[:, starts[k] * dim : (starts[k] + groups[k]) * dim],
                    in_=tiles[k][:],
                ).then_inc(out_sem, 16)
            s.wait_ge(out_sem, 16 * k_groups)

    return nc


def _make_in_maps(input_ids: np.ndarray, weight: np.ndarray):
    input_ids = np.asarray(input_ids)
    weight = np.ascontiguousarray(np.asarray(weight, dtype=np.float32))
    seq = input_ids.shape[1]
    q = seq // P
    in_maps = []
    for b in range(input_ids.shape[0]):
        ids_r = np.ascontiguousarray(input_ids[b].astype(np.int32).reshape(P, q))
        in_maps.append({"ids": ids_r, "weight": weight})
    return in_maps


def kernel(input_ids: np.ndarray, weight: np.ndarray) -> np.ndarray:
    input_ids = np.asarray(input_ids)
    B, S = input_ids.shape
    assert (B, S) == (BATCH, SEQ)

    nc = _build_nc()
    in_maps = _make_in_maps(input_ids, weight)
    res = run_bass_kernel_spmd(nc, in_maps, list(range(N_CORES)))
    return np.stack([res.results[b]["out"] for b in range(B)], axis=0)


# revision 16
# speedup vs baseline: 1.1406x; 1.0118x over previous
"""Embedding lookup (gather) on 8 Trainium2 NeuronCores.

Full inputs: input_ids [8, 4096] int32, weight [128000, 1024] f32.
Output: weight[input_ids] -> [8, 4096, 1024] f32.

Strategy: data-parallel over tokens. Core b handles batch row b
(4096 tokens). Each core receives the full weight table (host->HBM
staging is not timed) plus its ids reshaped to [128, 32] (token
p*32+g at ids[p, g]). On-device: 32 indirect-DMA gathers (one index
per partition each -- the HW DGE contract -- pulling weight rows
HBM -> SBUF), grouped into K store groups; each group's [128, m*1024]
tile is stored with one fully partition-contiguous DMA to the output.
Gathers run on gpsimd (SWDGE), stores on sync (HWDGE), overlapped via
per-group semaphores. No collectives needed.

Raw Bass (no TileContext): this walrus build rejects any instruction
carrying more than one sem-wait command, so waits are standalone
sequencer instructions; all sem waits are exact-total thresholds.
"""

from contextlib import ExitStack

import numpy as np

from concourse import bass, mybir
from concourse.bass_utils import run_bass_kernel_spmd

VOCAB = 128000
DIM = 1024
BATCH = 8
SEQ = 4096
N_CORES = 8
P = 128

Q = SEQ // P  # tokens per partition = gather ops per core (32)
# Store-group sizes (in gathers): small first groups start the write
# stream early; a tiny last group keeps the drain tail short.
GROUPS = (1, 1, 2, 2, 4, 4, 4, 4, 4, 4, 1, 1)
assert sum(GROUPS) == Q


def _build_nc(vocab=VOCAB, dim=DIM, seq=SEQ, groups=GROUPS):
    q = seq // P
    assert sum(groups) == q
    nc = bass.Bass()
    ids = nc.declare_dram_parameter("ids", [P, q], mybir.dt.int32, isOutput=False)
    weight = nc.declare_dram_parameter(
        "weight", [vocab, dim], mybir.dt.float32, isOutput=False
    )
    out = nc.declare_dram_parameter("out", [seq, dim], mybir.dt.float32, isOutput=True)
    # Output viewed per-partition: partition p's tokens are rows
    # [p*q, (p+1)*q), i.e. one contiguous q*dim chunk per partition.
    out_pview = out[:].rearrange("(p q) d -> p (q d)", p=P)

    k_groups = len(groups)
    starts = [sum(groups[:k]) for k in range(k_groups)]  # first gather of group k

    with ExitStack() as ctx:
        ids_tile = ctx.enter_context(nc.sbuf_tensor("ids_tile", [P, q], mybir.dt.int32))
        tiles = [
            ctx.enter_context(
                nc.sbuf_tensor(f"grp{k}", [P, groups[k] * dim], mybir.dt.float32)
            )
            for k in range(k_groups)
        ]
        ids_sem = ctx.enter_context(nc.semaphore("ids_sem"))
        gsems = [ctx.enter_context(nc.semaphore(f"gsem{k}")) for k in range(k_groups)]
        out_sem = ctx.enter_context(nc.semaphore("out_sem"))
        block = ctx.enter_context(nc.Block())

        @block.gpsimd
        def _(g):
            g.wait_ge(ids_sem, 16)
            for k in range(k_groups):
                for i in range(groups[k]):
                    j = starts[k] + i
                    g.indirect_dma_start(
                        out=tiles[k][:, i * dim : (i + 1) * dim],
                        out_offset=None,
                        in_=weight[:],
                        in_offset=bass.IndirectOffsetOnAxis(
                            ap=ids_tile[:, j : j + 1], axis=0
                        ),
                    ).then_inc(gsems[k], 16)

        @block.sync
        def _(s):
            # HWDGE ids load: Sync's preamble finishes earlier than GpSimd's
            # and HWDGE first-byte latency is lower, so the gathers unblock
            # sooner than with a gpsimd-issued load.
            s.dma_start(out=ids_tile[:], in_=ids[:]).then_inc(ids_sem, 16)
            for k in range(k_groups):
                # All gathers of group k done (exact total: groups[k]*16 incs).
                s.wait_ge(gsems[k], 16 * groups[k])
                s.dma_start(
                    out=out_pview[:, starts[k] * dim : (starts[k] + groups[k]) * dim],
                    in_=tiles[k][:],
                ).then_inc(out_sem, 16)
            s.wait_ge(out_sem, 16 * k_groups)

    return nc


def _make_in_maps(input_ids: np.ndarray, weight: np.ndarray):
    input_ids = np.asarray(input_ids)
    weight = np.ascontiguousarray(np.asarray(weight, dtype=np.float32))
    seq = input_ids.shape[1]
    q = seq // P
    in_maps = []
    for b in range(input_ids.shape[0]):
        ids_r = np.ascontiguousarray(input_ids[b].astype(np.int32).reshape(P, q))
        in_maps.append({"ids": ids_r, "weight": weight})
    return in_maps


def kernel(input_ids: np.ndarray, weight: np.ndarray) -> np.ndarray:
    input_ids = np.asarray(input_ids)
    B, S = input_ids.shape
    assert (B, S) == (BATCH, SEQ)

    in_maps = _make_in_maps(input_ids, weight)
    last_err = None
    for _attempt in range(2):
        try:
            nc = _build_nc()
            res = run_bass_kernel_spmd(nc, in_maps, list(range(N_CORES)))
            return np.stack([res.results[b]["out"] for b in range(B)], axis=0)
        except Exception as e:  # transient NRT device errors: retry once
            last_err = e
    raise last_err
